# revision 16
# baseline (speedup 1.0000x reference)
"""Multi-head causal attention kernel for 8 Trainium2 NeuronCores.

Problem: B=2, T=4096, D=512, H=8 (DH=64) fp32 MHA with causal mask.

Sharding: 16 (b, h) pairs -> 2 heads per core (core c: b = c//4, heads
2*(c%4), 2*(c%4)+1). Each core projects q/k into feature-major (DH x T)
layout and v into t-major fp8 (T x DH) layout from host-pre-transposed
bf16 inputs, runs causal flash-style attention per head, and applies
the output projection for its 2 heads producing a partial (T, D) bf16
output. The host sums the 4 partials per batch in f32 and adds bo.

Perf structure vs the naive version:
- Scores matmuls for the two heads run CONCURRENTLY in the PE via
  row-tiling (tile_position=(h*64, 0)) since DH=64 contraction only
  uses half the array rows.
- exp() is split between the Scalar (Act) engine (native Exp -> fp8)
  and the Vector (DVE) engine (Schraudolph bit-trick: one
  tensor_scalar computing round(s*log2e + B) into a uint8 view of an
  fp8e4 tile), balanced greedily at build time so neither engine
  serializes the PE. Boundary (causal-edge) tiles go to DVE where the
  mask is FUSED into the exp via scalar_tensor_tensor with a
  per-position bias tensor (B where visible, -1000 -> saturates to 0).
- AV uses one fp8 DoubleRow matmul per (key-block-pair, head):
  moving = p [128, 2, N] (the two key blocks interleaved in the free
  dim), stationary = v2 [128, 2, 80] (fp8, col 64 = ones so the
  rowsum rides along in psum row 64) -- half the PE streaming cost of
  the bf16 version.
- Raw-input DMA is batched 3 per t-block; output DMA is bf16 and
  issued from the Pool queue; input DMA from the Sync queue.
"""

import numpy as np

B, T, D, H = 2, 4096, 512, 8
DH = D // H          # 64
HPC = 2              # heads per core
NCORES = 8
QG = 512             # query-group width
NQG = T // QG        # 8
NT = T // 128        # 32 key tiles
NPAIR = NT // 2      # 16 key-block pairs
CCH = D // 128       # 4 contraction chunks for projections

WQK_COLS = 1024
WVO_COLS = 1536

# Schraudolph exp into bf16 bit pattern: bits16 = round(s*A16 + B16),
# written as int16 into a bf16-viewed tile (scale 1/8 folded into A16).
# Masked positions use bias -70000 -> int16 saturates to -32768 =
# 0x8000 = bf16 -0.0, which contributes exactly 0 to AV and rowsum.
A16 = 128 * 0.125 * 1.4426950408889634   # 23.083
B16 = 16250.25
MASKED_BIAS = -70000.0

LAST_EXEC_TIME_NS = None
LAST_RESULTS = None

# emit-time engine cost model (ns) for the Act/DVE/Pool balance
_ACT_NS = lambda cols: cols * 0.833 + 261.0
_DVE_NS = lambda cols: cols * 1.042 + 200.0
_POOL_NS = lambda cols: cols * 1.39 + 100.0


EXP_CHOICES = []  # (g, pair, h, engine) recorded at build time


def _build_module(with_qk_bias, with_v_bias, debug=False):
    import concourse.bacc as bacc
    import concourse.tile as tile
    from concourse import mybir
    from contextlib import ExitStack

    f32 = mybir.dt.float32
    bf16 = mybir.dt.bfloat16
    i16 = mybir.dt.int16
    EXP = mybir.ActivationFunctionType.Exp
    MULT = mybir.AluOpType.mult
    ADD = mybir.AluOpType.add

    nc = bacc.Bacc("TRN2", target_bir_lowering=False, debug=False)

    xqT = nc.dram_tensor("xqT", (D, T), bf16, kind="ExternalInput")
    xkT = nc.dram_tensor("xkT", (D, T), bf16, kind="ExternalInput")
    xvT = nc.dram_tensor("xvT", (D, T), bf16, kind="ExternalInput")
    wqk = nc.dram_tensor("wqk", (128, WQK_COLS), bf16, kind="ExternalInput")
    wvo = nc.dram_tensor("wvo", (128, WVO_COLS), bf16, kind="ExternalInput")
    # per-jj exp bias rows: B16 where visible, MASKED_BIAS where masked
    cmdram = nc.dram_tensor("cmdram", (128, 4, QG), f32, kind="ExternalInput")
    # plain 0/1 causal mask rows (bf16) for the Act-exp + Pool-mask path
    cm01dram = nc.dram_tensor("cm01dram", (128, 4, QG), bf16, kind="ExternalInput")
    bq2 = nc.dram_tensor("bq2", (HPC * DH, 1), f32, kind="ExternalInput")
    bk2 = nc.dram_tensor("bk2", (HPC * DH, 1), f32, kind="ExternalInput")
    bvr = nc.dram_tensor("bvr", (1, HPC * DH), bf16, kind="ExternalInput")
    out_part = nc.dram_tensor("out_part", (T, D), bf16, kind="ExternalOutput")
    if debug:
        dbg_q = nc.dram_tensor("dbg_q", (128, T), bf16, kind="ExternalOutput")
        dbg_k = nc.dram_tensor("dbg_k", (128, T), bf16, kind="ExternalOutput")
        dbg_v = nc.dram_tensor(
            "dbg_v", (128, NT * HPC * (DH + 1)), bf16,
            kind="ExternalOutput")
        dbg_rs = nc.dram_tensor("dbg_rs", (NQG * HPC, QG), f32,
                                kind="ExternalOutput")
        dbg_p = nc.dram_tensor("dbg_p", (2, 128, 2 * QG), bf16,
                               kind="ExternalOutput")

    EXP_CHOICES.clear()
    # build-time engine balance tallies
    tally = {"act": 0.0, "dve": 0.0, "pool": 0.0}

    with tile.TileContext(nc) as tc, ExitStack() as ctx:
        const = ctx.enter_context(tc.tile_pool(name="const", bufs=1))
        resid = ctx.enter_context(tc.tile_pool(name="resid", bufs=1))
        raws = ctx.enter_context(tc.tile_pool(name="raws", bufs=6))
        ppool = ctx.enter_context(tc.tile_pool(name="ppool", bufs=4))
        apool = ctx.enter_context(tc.tile_pool(name="apool", bufs=4))
        opool = ctx.enter_context(tc.tile_pool(name="opool", bufs=3))
        pscore = ctx.enter_context(tc.tile_pool(name="pscore", bufs=5, space="PSUM"))
        pmisc = ctx.enter_context(tc.tile_pool(name="pmisc", bufs=3, space="PSUM"))

        def copy_bal(dst, src, cols):
            """psum->sbuf copy on whichever of Act/DVE is less loaded."""
            if tally["act"] + _ACT_NS(cols) <= tally["dve"] + _DVE_NS(cols):
                tally["act"] += _ACT_NS(cols)
                nc.scalar.copy(dst, src)
            else:
                tally["dve"] += _DVE_NS(cols)
                nc.vector.tensor_copy(dst, src)

        # ---- constants ----
        wqk_sb = const.tile([128, WQK_COLS], bf16)
        nc.sync.dma_start(out=wqk_sb, in_=wqk[:])
        wq_sb = wqk_sb[:, 0:512].rearrange("p (c m) -> p c m", c=CCH)
        wk_sb = wqk_sb[:, 512:1024].rearrange("p (c m) -> p c m", c=CCH)

        # ---- residents ----
        qT_sb = resid.tile([HPC * DH, T], bf16)   # feature-major q, 2 heads
        kT_sb = resid.tile([HPC * DH, T], bf16)   # feature-major k, 2 heads
        # t-major v, per key-tile: [vA(64) | 1] [vB(64) | 1]; the ones
        # column makes the rowsum ride along in psum row 64 of av_ps
        v_sb = resid.tile([128, NT, HPC, DH + 1], bf16)
        nc.vector.memset(v_sb[:, :, :, DH], 1.0)

        # ---- emission helpers -------------------------------------------
        def emit_dma_block(tb):
            tiles = {}
            for key, src in (("k", xkT), ("q", xqT), ("v", xvT)):
                raw = raws.tile([128, CCH, QG], bf16, tag="raw", name="raw")
                nc.sync.dma_start(
                    out=raw,
                    in_=src[:, tb * QG:(tb + 1) * QG].rearrange(
                        "(c p) m -> p c m", c=CCH
                    ),
                )
                tiles[key] = raw
            return tiles

        def emit_proj(tb, rawt):
            for key, wsb, bias_sb, dst in (
                ("k", wk_sb, bk_sb, kT_sb),
                ("q", wq_sb, bq_sb, qT_sb),
            ):
                ps = pmisc.tile([128, QG], f32, tag="pm", name="ps_proj")
                for cc in range(CCH):
                    nc.tensor.matmul(
                        ps, wsb[:, cc, :], rawt[key][:, cc, :],
                        start=(cc == 0), stop=(cc == CCH - 1),
                    )
                if with_qk_bias:
                    tally["dve"] += _DVE_NS(QG)
                    nc.vector.tensor_scalar_add(
                        dst[:, tb * QG:(tb + 1) * QG], ps, bias_sb
                    )
                else:
                    copy_bal(dst[:, tb * QG:(tb + 1) * QG], ps, QG)
            for j in range(QG // 128):
                tt = tb * 4 + j
                ps = pmisc.tile([128, HPC * DH], f32, tag="pm", name="ps_v")
                for cc in range(CCH):
                    nc.tensor.matmul(
                        ps, rawt["v"][:, cc, j * 128:(j + 1) * 128],
                        wv_sb[:, cc, :],
                        start=(cc == 0),
                        stop=(cc == CCH - 1 and not with_v_bias),
                        skip_group_check=True,
                    )
                if with_v_bias:
                    nc.tensor.matmul(     # bias: out[t, d] += 1 * bv[d]
                        ps, ones1_sb, bvr_sb,
                        start=False, stop=True, skip_group_check=True,
                    )
                copy_bal(
                    v_sb[:, tt, :, 0:DH],
                    ps.rearrange("p (h d) -> p h d", h=HPC),
                    128,
                )

        def emit_scores(g, pair):
            # co0: first unmasked column (pair granularity) within group g
            q0 = g * QG
            co0 = max(0, (pair * 2 - 4 * g) * 128)
            boundary = pair >= 2 * g
            p_t = [
                ppool.tile([128, 2, QG], bf16, tag="p", name="p_t")
                for _ in range(HPC)
            ]
            ncols = QG - co0
            for i in range(2):
                kb = pair * 2 + i
                s_i = [
                    pscore.tile([128, QG], f32, tag="sc", name="s_ps")
                    for _ in range(HPC)
                ]
                for h in range(HPC):
                    nc.tensor.matmul(
                        s_i[h][:, co0:QG],
                        kT_sb[h * DH:(h + 1) * DH, kb * 128:(kb + 1) * 128],
                        qT_sb[h * DH:(h + 1) * DH, q0 + co0:q0 + QG],
                        start=True, stop=True,
                        tile_position=(h * DH, 0),
                    )
                for h in range(HPC):
                    dst = p_t[h][:, i, co0:QG]
                    srcp = s_i[h][:, co0:QG]
                    if boundary:
                        jj = kb - 4 * g
                        # plan A: DVE fused exp+mask; plan B: Act exp +
                        # mask-mul on Pool (or DVE)
                        c_dve = tally["dve"] + _DVE_NS(ncols)
                        mask_eng = ("pool"
                                    if tally["pool"] + _POOL_NS(ncols)
                                    <= tally["dve"] + _DVE_NS(ncols)
                                    else "dve")
                        c_act = max(
                            tally["act"] + _ACT_NS(ncols),
                            tally[mask_eng]
                            + (_POOL_NS(ncols) if mask_eng == "pool"
                               else _DVE_NS(ncols)),
                        )
                        if c_dve <= c_act:
                            tally["dve"] += _DVE_NS(ncols)
                            EXP_CHOICES.append((g, pair, i, h, "fused"))
                            nc.vector.scalar_tensor_tensor(
                                out=dst.bitcast(i16), in0=srcp,
                                scalar=A16,
                                in1=cmB_sb[:, jj, co0:QG],
                                op0=MULT, op1=ADD,
                            )
                        else:
                            tally["act"] += _ACT_NS(ncols)
                            EXP_CHOICES.append((g, pair, i, h, "act+" + mask_eng))
                            nc.scalar.activation(dst, srcp, EXP, scale=0.125)
                            if mask_eng == "pool":
                                tally["pool"] += _POOL_NS(ncols)
                                nc.gpsimd.tensor_mul(
                                    dst, dst, cm01_sb[:, jj, co0:QG]
                                )
                            else:
                                tally["dve"] += _DVE_NS(ncols)
                                nc.vector.tensor_mul(
                                    dst, dst, cm01_sb[:, jj, co0:QG]
                                )
                    else:
                        if (tally["act"] + _ACT_NS(ncols)
                                <= tally["dve"] + _DVE_NS(ncols)):
                            tally["act"] += _ACT_NS(ncols)
                            EXP_CHOICES.append((g, pair, i, h, "act"))
                            nc.scalar.activation(dst, srcp, EXP, scale=0.125)
                        else:
                            tally["dve"] += _DVE_NS(ncols)
                            EXP_CHOICES.append((g, pair, i, h, "dve"))
                            nc.vector.tensor_scalar(
                                out=dst.bitcast(i16), in0=srcp,
                                scalar1=A16, scalar2=B16, op0=MULT, op1=ADD,
                            )
            if debug and (g, pair) in ((6, 3), (6, 12)):
                slot = 0 if pair == 3 else 1
                nc.sync.dma_start(
                    out=dbg_p[slot].rearrange("p (i n) -> p i n", i=2),
                    in_=p_t[0],
                )
            return p_t, co0

        def make_av(g, pair, p_t, co0, av_ps):
            nkb = 4 * g + 4

            def emit_av():
                for i in range(2):
                    kb = pair * 2 + i
                    co = max(0, (kb - 4 * g) * 128)
                    for h in range(HPC):
                        nc.tensor.matmul(
                            av_ps[h][:, co:QG], v_sb[:, kb, h, :],
                            p_t[h][:, i, co:QG],
                            start=(kb == 0), stop=(kb == nkb - 1),
                            skip_group_check=True,
                        )
            return emit_av

        def make_norm(g, av_ps):
            def emit_norm():
                attn = []
                for h in range(HPC):
                    rs = apool.tile([1, QG], f32, tag="rs", name="rs")
                    copy_bal(rs, av_ps[h][DH:DH + 1, :], QG)
                    if debug:
                        nc.sync.dma_start(
                            out=dbg_rs[g * HPC + h:g * HPC + h + 1, :], in_=rs
                        )
                    rec = apool.tile([1, QG], f32, tag="rec", name="rec")
                    tally["dve"] += _DVE_NS(QG)
                    nc.vector.reciprocal_approx_fast(rec, rs)
                    rb = apool.tile([DH, QG], f32, tag="rb", name="rb")
                    nc.gpsimd.partition_broadcast(rb, rec)
                    at = apool.tile([DH, QG], bf16, tag="at", name="at")
                    tally["dve"] += _DVE_NS(QG)
                    nc.vector.tensor_mul(at, av_ps[h][0:DH, :], rb)
                    attn.append(at)
                return attn
            return emit_norm

        def make_oproj(g, attn):
            q0 = g * QG

            def emit_oproj():
                for j in range(QG // 128):
                    o_ps = pmisc.tile([128, D], f32, tag="pm", name="o_ps")
                    nc.tensor.matmul(
                        o_ps, attn[0][:, j * 128:(j + 1) * 128], woa_sb,
                        start=True, stop=False, skip_group_check=True,
                    )
                    nc.tensor.matmul(
                        o_ps, attn[1][:, j * 128:(j + 1) * 128], wob_sb,
                        start=False, stop=True, skip_group_check=True,
                    )
                    ot = opool.tile([128, D], bf16, tag="ot", name="ot")
                    copy_bal(ot, o_ps, D)
                    nc.gpsimd.dma_start(
                        out=out_part[q0 + j * 128:q0 + (j + 1) * 128, :], in_=ot
                    )
            return emit_oproj

        # ---- main interleaved loop --------------------------------------
        rawt = emit_dma_block(0)
        wvo_sb = const.tile([128, WVO_COLS], bf16)
        nc.sync.dma_start(out=wvo_sb, in_=wvo[:])
        wv_sb = wvo_sb[:, 0:512].rearrange("p (c m) -> p c m", c=CCH)
        woa_sb = wvo_sb[0:DH, 512:1024]                      # [64, 512]
        wob_sb = wvo_sb[0:DH, 1024:1536]                     # [64, 512]
        cmB_sb = const.tile([128, 4, QG], f32, name="cmB_sb")
        nc.sync.dma_start(out=cmB_sb, in_=cmdram[:])
        cm01_sb = const.tile([128, 4, QG], bf16, name="cm01_sb")
        nc.sync.dma_start(out=cm01_sb, in_=cm01dram[:])
        bq_sb = bk_sb = bvr_sb = ones1_sb = None
        if with_qk_bias:
            bq_sb = const.tile([HPC * DH, 1], f32)
            nc.sync.dma_start(out=bq_sb, in_=bq2[:])
            bk_sb = const.tile([HPC * DH, 1], f32)
            nc.sync.dma_start(out=bk_sb, in_=bk2[:])
        if with_v_bias:
            bvr_sb = const.tile([1, HPC * DH], bf16)
            nc.sync.dma_start(out=bvr_sb, in_=bvr[:])
            ones1_sb = const.tile([1, 128], bf16)
            nc.vector.memset(ones1_sb, 1.0)

        prev_av = None
        pend_norm = None
        pend_oproj_mk = None
        for g in range(NQG):
            emit_proj(g, rawt)
            if g + 1 < NQG:
                rawt = emit_dma_block(g + 1)
            av_ps = [
                pmisc.tile([DH + 1, QG], f32, tag="pm", name="av_ps")
                for _ in range(HPC)
            ]
            for pair in range(2 * g + 2):
                p_t, co0 = emit_scores(g, pair)
                if prev_av is not None:
                    prev_av()
                if pend_norm is not None:
                    attn_prev = pend_norm()
                    pend_oproj_mk = make_oproj(g - 1, attn_prev)
                    pend_norm = None
                elif pend_oproj_mk is not None:
                    pend_oproj_mk()
                    pend_oproj_mk = None
                prev_av = make_av(g, pair, p_t, co0, av_ps)
            pend_norm = make_norm(g, av_ps)
        prev_av()
        attn_last = pend_norm()
        make_oproj(NQG - 1, attn_last)()
        if debug:
            nc.sync.dma_start(out=dbg_q[:], in_=qT_sb)
            nc.sync.dma_start(out=dbg_k[:], in_=kT_sb)
            nc.sync.dma_start(
                out=dbg_v[:].rearrange(
                    "p (a b c) -> p a b c", a=NT, b=HPC
                ),
                in_=v_sb,
            )

    nc.compile()
    return nc


def _numpy_reference(query, key, value, mask, Wq, bq, Wk, bk, Wv, bv, Wo, bo):
    def split_heads(x):
        b, t, d = x.shape
        return x.reshape(b, t, H, DH).transpose(0, 2, 1, 3)

    q = split_heads(query @ Wq.T + bq)
    k = split_heads(key @ Wk.T + bk)
    v = split_heads(value @ Wv.T + bv)
    scale = 1.0 / np.sqrt(np.float32(DH))
    out = np.empty((B, H, T, DH), np.float32)
    for b in range(B):
        for h in range(H):
            s = (q[b, h] @ k[b, h].T) * scale
            s = np.where(mask[b] == 0, -np.inf, s)
            s = s - s.max(axis=-1, keepdims=True)
            p = np.exp(s)
            p /= p.sum(axis=-1, keepdims=True)
            out[b, h] = p @ v[b, h]
    out = out.transpose(0, 2, 1, 3).reshape(B, T, D)
    return out @ Wo.T + bo


def kernel(query, key, value, mask, Wq, bq, Wk, bk, Wv, bv, Wo, bo):
    global LAST_EXEC_TIME_NS, LAST_RESULTS
    import ml_dtypes

    bfloat16 = ml_dtypes.bfloat16
    query = np.asarray(query, np.float32)
    key = np.asarray(key, np.float32)
    value = np.asarray(value, np.float32)
    mask = np.asarray(mask)
    Wq, bq = np.asarray(Wq, np.float32), np.asarray(bq, np.float32)
    Wk, bk = np.asarray(Wk, np.float32), np.asarray(bk, np.float32)
    Wv, bv = np.asarray(Wv, np.float32), np.asarray(bv, np.float32)
    Wo, bo = np.asarray(Wo, np.float32), np.asarray(bo, np.float32)

    tril = np.tril(np.ones((T, T), mask.dtype))
    causal = all(np.array_equal(mask[b], tril) for b in range(B))
    if not causal:
        return _numpy_reference(
            query, key, value, mask, Wq, bq, Wk, bk, Wv, bv, Wo, bo
        ).astype(np.float32)

    # exp bias rows: B16 where column visible (c >= 128*jj + r), else masked
    r = np.arange(128, dtype=np.int64)[:, None]
    c = np.arange(QG, dtype=np.int64)[None, :]
    cmB = np.stack(
        [np.where(c >= 128 * j + r, B16, MASKED_BIAS).astype(np.float32)
         for j in range(4)], axis=1,
    )  # (128, 4, QG)
    cm01 = np.stack(
        [(c >= 128 * j + r).astype(bfloat16) for j in range(4)], axis=1,
    )  # (128, 4, QG)

    with_qk_bias = bool(np.any(bq != 0) or np.any(bk != 0))
    with_v_bias = bool(np.any(bv != 0))

    in_maps = []
    for core in range(NCORES):
        b = core // 4
        h0 = (core % 4) * HPC
        sl = slice(h0 * DH, (h0 + HPC) * DH)
        wq_r = np.ascontiguousarray(Wq[sl, :].T).reshape(CCH, 128, 128).transpose(1, 0, 2).reshape(128, 512)
        wk_r = np.ascontiguousarray(Wk[sl, :].T).reshape(CCH, 128, 128).transpose(1, 0, 2).reshape(128, 512)
        wv_r = np.ascontiguousarray(Wv[sl, :].T).reshape(CCH, 128, 128).transpose(1, 0, 2).reshape(128, 512)
        wo_r = np.zeros((128, 1024), np.float32)
        wo_r[0:DH, 0:512] = Wo[:, h0 * DH:(h0 + 1) * DH].T
        wo_r[0:DH, 512:1024] = Wo[:, (h0 + 1) * DH:(h0 + 2) * DH].T
        in_maps.append({
            "xqT": np.ascontiguousarray(query[b].T).astype(bfloat16),
            "xkT": np.ascontiguousarray(key[b].T).astype(bfloat16),
            "xvT": np.ascontiguousarray(value[b].T).astype(bfloat16),
            "wqk": np.concatenate([wq_r, wk_r], axis=1).astype(bfloat16),
            "wvo": np.concatenate([wv_r, wo_r], axis=1).astype(bfloat16),
            "cmdram": cmB,
            "cm01dram": cm01,
            "bq2": np.ascontiguousarray(bq[sl].reshape(HPC * DH, 1)),
            "bk2": np.ascontiguousarray(bk[sl].reshape(HPC * DH, 1)),
            "bvr": bv[sl].reshape(1, HPC * DH).astype(bfloat16),
        })

    import os

    debug = os.environ.get("KERNEL_DEBUG", "0") == "1"
    nc = _build_module(with_qk_bias, with_v_bias, debug=debug)
    from concourse import bass_utils

    trace = os.environ.get("KERNEL_TRACE", "0") == "1"
    res = bass_utils.run_bass_kernel_spmd(
        nc, in_maps, core_ids=list(range(NCORES)), trace=trace
    )
    LAST_RESULTS = res
    LAST_EXEC_TIME_NS = res.exec_time_ns

    out = np.zeros((B, T, D), np.float32)
    for core in range(NCORES):
        out[core // 4] += np.asarray(res.results[core]["out_part"], np.float32)
    out += bo[None, None, :]
    return out


# revision 17
# speedup vs baseline: 1.1030x; 1.1030x over previous
"""Multi-head causal attention kernel for 8 Trainium2 NeuronCores.

Problem: B=2, T=4096, D=512, H=8 (DH=64) fp32 MHA with causal mask.

Sharding: 16 (b, h) pairs -> 2 heads per core (core c: b = c//4, heads
2*(c%4), 2*(c%4)+1). Each core projects q/k into feature-major (DH x T)
layout and v into t-major fp8 (T x DH) layout from host-pre-transposed
bf16 inputs, runs causal flash-style attention per head, and applies
the output projection for its 2 heads producing a partial (T, D) bf16
output. The host sums the 4 partials per batch in f32 and adds bo.

Perf structure vs the naive version:
- Scores matmuls for the two heads run CONCURRENTLY in the PE via
  row-tiling (tile_position=(h*64, 0)) since DH=64 contraction only
  uses half the array rows.
- exp() is split between the Scalar (Act) engine (native Exp -> fp8)
  and the Vector (DVE) engine (Schraudolph bit-trick: one
  tensor_scalar computing round(s*log2e + B) into a uint8 view of an
  fp8e4 tile), balanced greedily at build time so neither engine
  serializes the PE. Boundary (causal-edge) tiles go to DVE where the
  mask is FUSED into the exp via scalar_tensor_tensor with a
  per-position bias tensor (B where visible, -1000 -> saturates to 0).
- AV uses one fp8 DoubleRow matmul per (key-block-pair, head):
  moving = p [128, 2, N] (the two key blocks interleaved in the free
  dim), stationary = v2 [128, 2, 80] (fp8, col 64 = ones so the
  rowsum rides along in psum row 64) -- half the PE streaming cost of
  the bf16 version.
- Raw-input DMA is batched 3 per t-block; output DMA is bf16 and
  issued from the Pool queue; input DMA from the Sync queue.
"""

import numpy as np

B, T, D, H = 2, 4096, 512, 8
DH = D // H          # 64
HPC = 2              # heads per core
NCORES = 8
QG = 512             # query-group width
NQG = T // QG        # 8
NT = T // 128        # 32 key tiles
NPAIR = NT // 2      # 16 key-block pairs
CCH = D // 128       # 4 contraction chunks for projections

WQK_COLS = 1024
WVO_COLS = 1536

# Schraudolph exp into bf16 bit pattern: bits16 = round(s*A16 + B16),
# written as int16 into a bf16-viewed tile (scale 1/8 folded into A16).
# Masked positions use bias -70000 -> int16 saturates to -32768 =
# 0x8000 = bf16 -0.0, which contributes exactly 0 to AV and rowsum.
A16 = 128 * 0.125 * 1.4426950408889634   # 23.083
B16 = 16250.25
MASKED_BIAS = -70000.0

LAST_EXEC_TIME_NS = None
LAST_RESULTS = None

# emit-time engine cost model (ns) for the Act/DVE/Pool balance
_ACT_NS = lambda cols: cols * 0.833 + 261.0
_DVE_NS = lambda cols: cols * 1.042 + 200.0
_POOL_NS = lambda cols: cols * 1.39 + 100.0


EXP_CHOICES = []  # (g, pair, h, engine) recorded at build time


def _build_module(with_qk_bias, with_v_bias, debug=False):
    import concourse.bacc as bacc
    import concourse.tile as tile
    from concourse import mybir
    from contextlib import ExitStack

    f32 = mybir.dt.float32
    bf16 = mybir.dt.bfloat16
    i16 = mybir.dt.int16
    EXP = mybir.ActivationFunctionType.Exp
    MULT = mybir.AluOpType.mult
    ADD = mybir.AluOpType.add

    nc = bacc.Bacc("TRN2", target_bir_lowering=False, debug=False)

    xqT = nc.dram_tensor("xqT", (D, T), bf16, kind="ExternalInput")
    xkT = nc.dram_tensor("xkT", (D, T), bf16, kind="ExternalInput")
    xvT = nc.dram_tensor("xvT", (D, T), bf16, kind="ExternalInput")
    wqk = nc.dram_tensor("wqk", (128, WQK_COLS), bf16, kind="ExternalInput")
    wvo = nc.dram_tensor("wvo", (128, WVO_COLS), bf16, kind="ExternalInput")
    # per-jj exp bias rows: B16 where visible, MASKED_BIAS where masked
    cmdram = nc.dram_tensor("cmdram", (128, 4, QG), f32, kind="ExternalInput")
    # plain 0/1 causal mask rows (bf16) for the Act-exp + Pool-mask path
    cm01dram = nc.dram_tensor("cm01dram", (128, 4, QG), bf16, kind="ExternalInput")
    bq2 = nc.dram_tensor("bq2", (HPC * DH, 1), f32, kind="ExternalInput")
    bk2 = nc.dram_tensor("bk2", (HPC * DH, 1), f32, kind="ExternalInput")
    bvr = nc.dram_tensor("bvr", (1, HPC * DH), bf16, kind="ExternalInput")
    out_part = nc.dram_tensor("out_part", (T, D), bf16, kind="ExternalOutput")
    if debug:
        dbg_q = nc.dram_tensor("dbg_q", (128, T), bf16, kind="ExternalOutput")
        dbg_k = nc.dram_tensor("dbg_k", (128, T), bf16, kind="ExternalOutput")
        dbg_v = nc.dram_tensor(
            "dbg_v", (128, NT * HPC * (DH + 1)), bf16,
            kind="ExternalOutput")
        dbg_rs = nc.dram_tensor("dbg_rs", (NQG * HPC, QG), f32,
                                kind="ExternalOutput")
        dbg_p = nc.dram_tensor("dbg_p", (2, 128, 2 * QG), bf16,
                               kind="ExternalOutput")

    EXP_CHOICES.clear()
    # build-time engine balance tallies
    tally = {"act": 0.0, "dve": 0.0, "pool": 0.0}

    with tile.TileContext(nc) as tc, ExitStack() as ctx:
        const = ctx.enter_context(tc.tile_pool(name="const", bufs=1))
        resid = ctx.enter_context(tc.tile_pool(name="resid", bufs=1))
        raws = ctx.enter_context(tc.tile_pool(name="raws", bufs=6))
        ppool = ctx.enter_context(tc.tile_pool(name="ppool", bufs=4))
        apool = ctx.enter_context(tc.tile_pool(name="apool", bufs=4))
        opool = ctx.enter_context(tc.tile_pool(name="opool", bufs=3))
        pscore = ctx.enter_context(tc.tile_pool(name="pscore", bufs=4, space="PSUM"))
        pmisc = ctx.enter_context(tc.tile_pool(name="pmisc", bufs=4, space="PSUM"))

        def copy_bal(dst, src, cols):
            """psum->sbuf copy on whichever of Act/DVE is less loaded."""
            if tally["act"] + _ACT_NS(cols) <= tally["dve"] + _DVE_NS(cols):
                tally["act"] += _ACT_NS(cols)
                nc.scalar.copy(dst, src)
            else:
                tally["dve"] += _DVE_NS(cols)
                nc.vector.tensor_copy(dst, src)

        # ---- constants ----
        wqk_sb = const.tile([128, WQK_COLS], bf16)
        nc.sync.dma_start(out=wqk_sb, in_=wqk[:])
        wq_sb = wqk_sb[:, 0:512].rearrange("p (c m) -> p c m", c=CCH)
        wk_sb = wqk_sb[:, 512:1024].rearrange("p (c m) -> p c m", c=CCH)

        # ---- residents ----
        qT_sb = resid.tile([HPC * DH, T], bf16)   # feature-major q, 2 heads
        kT_sb = resid.tile([HPC * DH, T], bf16)   # feature-major k, 2 heads
        # t-major v, per key-tile: [vA(64) | 1] [vB(64) | 1]; the ones
        # column makes the rowsum ride along in psum row 64 of av_ps
        v_sb = resid.tile([128, NT, HPC, DH + 1], bf16)
        nc.vector.memset(v_sb[:, :, :, DH], 1.0)

        # ---- emission helpers -------------------------------------------
        def emit_dma_block(tb):
            tiles = {}
            for key, src in (("k", xkT), ("q", xqT), ("v", xvT)):
                raw = raws.tile([128, CCH, QG], bf16, tag="raw", name="raw")
                nc.sync.dma_start(
                    out=raw,
                    in_=src[:, tb * QG:(tb + 1) * QG].rearrange(
                        "(c p) m -> p c m", c=CCH
                    ),
                )
                tiles[key] = raw
            return tiles

        def emit_proj(tb, rawt):
            for key, wsb, bias_sb, dst in (
                ("k", wk_sb, bk_sb, kT_sb),
                ("q", wq_sb, bq_sb, qT_sb),
            ):
                ps = pmisc.tile([128, QG], f32, tag="pm", name="ps_proj")
                for cc in range(CCH):
                    nc.tensor.matmul(
                        ps, wsb[:, cc, :], rawt[key][:, cc, :],
                        start=(cc == 0), stop=(cc == CCH - 1),
                    )
                if with_qk_bias:
                    tally["dve"] += _DVE_NS(QG)
                    nc.vector.tensor_scalar_add(
                        dst[:, tb * QG:(tb + 1) * QG], ps, bias_sb
                    )
                else:
                    copy_bal(dst[:, tb * QG:(tb + 1) * QG], ps, QG)
            for j in range(QG // 128):
                tt = tb * 4 + j
                ps = pmisc.tile([128, HPC * DH], f32, tag="pm", name="ps_v")
                for cc in range(CCH):
                    nc.tensor.matmul(
                        ps, rawt["v"][:, cc, j * 128:(j + 1) * 128],
                        wv_sb[:, cc, :],
                        start=(cc == 0),
                        stop=(cc == CCH - 1 and not with_v_bias),
                        skip_group_check=True,
                    )
                if with_v_bias:
                    nc.tensor.matmul(     # bias: out[t, d] += 1 * bv[d]
                        ps, ones1_sb, bvr_sb,
                        start=False, stop=True, skip_group_check=True,
                    )
                copy_bal(
                    v_sb[:, tt, :, 0:DH],
                    ps.rearrange("p (h d) -> p h d", h=HPC),
                    128,
                )

        def emit_scores(g, pair):
            # co0: first unmasked column (pair granularity) within group g
            q0 = g * QG
            co0 = max(0, (pair * 2 - 4 * g) * 128)
            boundary = pair >= 2 * g
            p_t = [
                ppool.tile([128, 2, QG], bf16, tag="p", name="p_t")
                for _ in range(HPC)
            ]
            ncols = QG - co0
            for i in range(2):
                kb = pair * 2 + i
                s_i = [
                    pscore.tile([128, QG], f32, tag="sc", name="s_ps")
                    for _ in range(HPC)
                ]
                for h in range(HPC):
                    nc.tensor.matmul(
                        s_i[h][:, co0:QG],
                        kT_sb[h * DH:(h + 1) * DH, kb * 128:(kb + 1) * 128],
                        qT_sb[h * DH:(h + 1) * DH, q0 + co0:q0 + QG],
                        start=True, stop=True,
                        tile_position=(h * DH, 0),
                    )
                for h in range(HPC):
                    dst = p_t[h][:, i, co0:QG]
                    srcp = s_i[h][:, co0:QG]
                    if boundary:
                        jj = kb - 4 * g
                        # plan A: DVE fused exp+mask; plan B: Act exp +
                        # mask-mul on Pool (or DVE)
                        c_dve = tally["dve"] + _DVE_NS(ncols)
                        mask_eng = ("pool"
                                    if tally["pool"] + _POOL_NS(ncols)
                                    <= tally["dve"] + _DVE_NS(ncols)
                                    else "dve")
                        c_act = max(
                            tally["act"] + _ACT_NS(ncols),
                            tally[mask_eng]
                            + (_POOL_NS(ncols) if mask_eng == "pool"
                               else _DVE_NS(ncols)),
                        )
                        if c_dve <= c_act:
                            tally["dve"] += _DVE_NS(ncols)
                            EXP_CHOICES.append((g, pair, i, h, "fused"))
                            nc.vector.scalar_tensor_tensor(
                                out=dst.bitcast(i16), in0=srcp,
                                scalar=A16,
                                in1=cmB_sb[:, jj, co0:QG],
                                op0=MULT, op1=ADD,
                            )
                        else:
                            tally["act"] += _ACT_NS(ncols)
                            EXP_CHOICES.append((g, pair, i, h, "act+" + mask_eng))
                            nc.scalar.activation(dst, srcp, EXP, scale=0.125)
                            if mask_eng == "pool":
                                tally["pool"] += _POOL_NS(ncols)
                                nc.gpsimd.tensor_mul(
                                    dst, dst, cm01_sb[:, jj, co0:QG]
                                )
                            else:
                                tally["dve"] += _DVE_NS(ncols)
                                nc.vector.tensor_mul(
                                    dst, dst, cm01_sb[:, jj, co0:QG]
                                )
                    else:
                        if (tally["act"] + _ACT_NS(ncols)
                                <= tally["dve"] + _DVE_NS(ncols)):
                            tally["act"] += _ACT_NS(ncols)
                            EXP_CHOICES.append((g, pair, i, h, "act"))
                            nc.scalar.activation(dst, srcp, EXP, scale=0.125)
                        else:
                            tally["dve"] += _DVE_NS(ncols)
                            EXP_CHOICES.append((g, pair, i, h, "dve"))
                            nc.vector.tensor_scalar(
                                out=dst.bitcast(i16), in0=srcp,
                                scalar1=A16, scalar2=B16, op0=MULT, op1=ADD,
                            )
            if debug and (g, pair) in ((6, 3), (6, 12)):
                slot = 0 if pair == 3 else 1
                nc.sync.dma_start(
                    out=dbg_p[slot].rearrange("p (i n) -> p i n", i=2),
                    in_=p_t[0],
                )
            return p_t, co0

        def make_av(g, pair, p_t, co0, av_ps):
            nkb = 4 * g + 4

            def emit_av():
                for i in range(2):
                    kb = pair * 2 + i
                    co = max(0, (kb - 4 * g) * 128)
                    for h in range(HPC):
                        nc.tensor.matmul(
                            av_ps[h][:, co:QG], v_sb[:, kb, h, :],
                            p_t[h][:, i, co:QG],
                            start=(kb == 0), stop=(kb == nkb - 1),
                            skip_group_check=True,
                        )
            return emit_av

        def make_norm(g, av_ps):
            def emit_norm():
                attn = []
                for h in range(HPC):
                    rs = apool.tile([1, QG], f32, tag="rs", name="rs")
                    copy_bal(rs, av_ps[h][DH:DH + 1, :], QG)
                    if debug:
                        nc.sync.dma_start(
                            out=dbg_rs[g * HPC + h:g * HPC + h + 1, :], in_=rs
                        )
                    rec = apool.tile([1, QG], f32, tag="rec", name="rec")
                    tally["dve"] += _DVE_NS(QG)
                    nc.vector.reciprocal_approx_fast(rec, rs)
                    rb = apool.tile([DH, QG], f32, tag="rb", name="rb")
                    nc.gpsimd.partition_broadcast(rb, rec)
                    at = apool.tile([DH, QG], bf16, tag="at", name="at")
                    tally["dve"] += _DVE_NS(QG)
                    nc.vector.tensor_mul(at, av_ps[h][0:DH, :], rb)
                    attn.append(at)
                return attn
            return emit_norm

        def make_oproj(g, attn):
            q0 = g * QG

            def emit_oproj():
                for j in range(QG // 128):
                    o_ps = pmisc.tile([128, D], f32, tag="pm", name="o_ps")
                    nc.tensor.matmul(
                        o_ps, attn[0][:, j * 128:(j + 1) * 128], woa_sb,
                        start=True, stop=False, skip_group_check=True,
                    )
                    nc.tensor.matmul(
                        o_ps, attn[1][:, j * 128:(j + 1) * 128], wob_sb,
                        start=False, stop=True, skip_group_check=True,
                    )
                    ot = opool.tile([128, D], bf16, tag="ot", name="ot")
                    copy_bal(ot, o_ps, D)
                    nc.gpsimd.dma_start(
                        out=out_part[q0 + j * 128:q0 + (j + 1) * 128, :], in_=ot
                    )
            return emit_oproj

        # ---- main interleaved loop --------------------------------------
        rawt = emit_dma_block(0)
        wvo_sb = const.tile([128, WVO_COLS], bf16)
        nc.sync.dma_start(out=wvo_sb, in_=wvo[:])
        wv_sb = wvo_sb[:, 0:512].rearrange("p (c m) -> p c m", c=CCH)
        woa_sb = wvo_sb[0:DH, 512:1024]                      # [64, 512]
        wob_sb = wvo_sb[0:DH, 1024:1536]                     # [64, 512]
        cmB_sb = const.tile([128, 4, QG], f32, name="cmB_sb")
        nc.sync.dma_start(out=cmB_sb, in_=cmdram[:])
        cm01_sb = const.tile([128, 4, QG], bf16, name="cm01_sb")
        nc.sync.dma_start(out=cm01_sb, in_=cm01dram[:])
        bq_sb = bk_sb = bvr_sb = ones1_sb = None
        if with_qk_bias:
            bq_sb = const.tile([HPC * DH, 1], f32)
            nc.sync.dma_start(out=bq_sb, in_=bq2[:])
            bk_sb = const.tile([HPC * DH, 1], f32)
            nc.sync.dma_start(out=bk_sb, in_=bk2[:])
        if with_v_bias:
            bvr_sb = const.tile([1, HPC * DH], bf16)
            nc.sync.dma_start(out=bvr_sb, in_=bvr[:])
            ones1_sb = const.tile([1, 128], bf16)
            nc.vector.memset(ones1_sb, 1.0)

        prev_av = None
        pend_norm = None
        pend_oproj_mk = None
        for g in range(NQG):
            emit_proj(g, rawt)
            if g + 1 < NQG:
                rawt = emit_dma_block(g + 1)
            av_ps = [
                pmisc.tile([DH + 1, QG], f32, tag="pm", name="av_ps")
                for _ in range(HPC)
            ]
            for pair in range(2 * g + 2):
                p_t, co0 = emit_scores(g, pair)
                if prev_av is not None:
                    prev_av()
                if pend_norm is not None:
                    attn_prev = pend_norm()
                    pend_oproj_mk = make_oproj(g - 1, attn_prev)
                    pend_norm = None
                elif pend_oproj_mk is not None:
                    pend_oproj_mk()
                    pend_oproj_mk = None
                prev_av = make_av(g, pair, p_t, co0, av_ps)
            pend_norm = make_norm(g, av_ps)
        prev_av()
        attn_last = pend_norm()
        make_oproj(NQG - 1, attn_last)()
        if debug:
            nc.sync.dma_start(out=dbg_q[:], in_=qT_sb)
            nc.sync.dma_start(out=dbg_k[:], in_=kT_sb)
            nc.sync.dma_start(
                out=dbg_v[:].rearrange(
                    "p (a b c) -> p a b c", a=NT, b=HPC
                ),
                in_=v_sb,
            )

    nc.compile()
    return nc


def _numpy_reference(query, key, value, mask, Wq, bq, Wk, bk, Wv, bv, Wo, bo):
    def split_heads(x):
        b, t, d = x.shape
        return x.reshape(b, t, H, DH).transpose(0, 2, 1, 3)

    q = split_heads(query @ Wq.T + bq)
    k = split_heads(key @ Wk.T + bk)
    v = split_heads(value @ Wv.T + bv)
    scale = 1.0 / np.sqrt(np.float32(DH))
    out = np.empty((B, H, T, DH), np.float32)
    for b in range(B):
        for h in range(H):
            s = (q[b, h] @ k[b, h].T) * scale
            s = np.where(mask[b] == 0, -np.inf, s)
            s = s - s.max(axis=-1, keepdims=True)
            p = np.exp(s)
            p /= p.sum(axis=-1, keepdims=True)
            out[b, h] = p @ v[b, h]
    out = out.transpose(0, 2, 1, 3).reshape(B, T, D)
    return out @ Wo.T + bo


def kernel(query, key, value, mask, Wq, bq, Wk, bk, Wv, bv, Wo, bo):
    global LAST_EXEC_TIME_NS, LAST_RESULTS
    import ml_dtypes

    bfloat16 = ml_dtypes.bfloat16
    query = np.asarray(query, np.float32)
    key = np.asarray(key, np.float32)
    value = np.asarray(value, np.float32)
    mask = np.asarray(mask)
    Wq, bq = np.asarray(Wq, np.float32), np.asarray(bq, np.float32)
    Wk, bk = np.asarray(Wk, np.float32), np.asarray(bk, np.float32)
    Wv, bv = np.asarray(Wv, np.float32), np.asarray(bv, np.float32)
    Wo, bo = np.asarray(Wo, np.float32), np.asarray(bo, np.float32)

    tril = np.tril(np.ones((T, T), mask.dtype))
    causal = all(np.array_equal(mask[b], tril) for b in range(B))
    if not causal:
        return _numpy_reference(
            query, key, value, mask, Wq, bq, Wk, bk, Wv, bv, Wo, bo
        ).astype(np.float32)

    # exp bias rows: B16 where column visible (c >= 128*jj + r), else masked
    r = np.arange(128, dtype=np.int64)[:, None]
    c = np.arange(QG, dtype=np.int64)[None, :]
    cmB = np.stack(
        [np.where(c >= 128 * j + r, B16, MASKED_BIAS).astype(np.float32)
         for j in range(4)], axis=1,
    )  # (128, 4, QG)
    cm01 = np.stack(
        [(c >= 128 * j + r).astype(bfloat16) for j in range(4)], axis=1,
    )  # (128, 4, QG)

    with_qk_bias = bool(np.any(bq != 0) or np.any(bk != 0))
    with_v_bias = bool(np.any(bv != 0))

    in_maps = []
    for core in range(NCORES):
        b = core // 4
        h0 = (core % 4) * HPC
        sl = slice(h0 * DH, (h0 + HPC) * DH)
        wq_r = np.ascontiguousarray(Wq[sl, :].T).reshape(CCH, 128, 128).transpose(1, 0, 2).reshape(128, 512)
        wk_r = np.ascontiguousarray(Wk[sl, :].T).reshape(CCH, 128, 128).transpose(1, 0, 2).reshape(128, 512)
        wv_r = np.ascontiguousarray(Wv[sl, :].T).reshape(CCH, 128, 128).transpose(1, 0, 2).reshape(128, 512)
        wo_r = np.zeros((128, 1024), np.float32)
        wo_r[0:DH, 0:512] = Wo[:, h0 * DH:(h0 + 1) * DH].T
        wo_r[0:DH, 512:1024] = Wo[:, (h0 + 1) * DH:(h0 + 2) * DH].T
        in_maps.append({
            "xqT": np.ascontiguousarray(query[b].T).astype(bfloat16),
            "xkT": np.ascontiguousarray(key[b].T).astype(bfloat16),
            "xvT": np.ascontiguousarray(value[b].T).astype(bfloat16),
            "wqk": np.concatenate([wq_r, wk_r], axis=1).astype(bfloat16),
            "wvo": np.concatenate([wv_r, wo_r], axis=1).astype(bfloat16),
            "cmdram": cmB,
            "cm01dram": cm01,
            "bq2": np.ascontiguousarray(bq[sl].reshape(HPC * DH, 1)),
            "bk2": np.ascontiguousarray(bk[sl].reshape(HPC * DH, 1)),
            "bvr": bv[sl].reshape(1, HPC * DH).astype(bfloat16),
        })

    import os

    debug = os.environ.get("KERNEL_DEBUG", "0") == "1"
    nc = _build_module(with_qk_bias, with_v_bias, debug=debug)
    from concourse import bass_utils

    trace = os.environ.get("KERNEL_TRACE", "0") == "1"
    res = bass_utils.run_bass_kernel_spmd(
        nc, in_maps, core_ids=list(range(NCORES)), trace=trace
    )
    LAST_RESULTS = res
    LAST_EXEC_TIME_NS = res.exec_time_ns

    out = np.zeros((B, T, D), np.float32)
    for core in range(NCORES):
        out[core // 4] += np.asarray(res.results[core]["out_part"], np.float32)
    out += bo[None, None, :]
    return out
